# revision 35
# baseline (speedup 1.0000x reference)
"""BatchAllTripletLoss on 8 Trainium2 NeuronCores via Bass/Tile.

Math: for anchors i, positives j (same label, j!=i), negatives k (diff label):
  total        = sum_{i,j,k} relu(d_ij - d_ik + margin)
  num_non_easy = #{(i,j,k): d_ik < d_ij + margin}
  loss         = total / num_non_easy ; frac = num_non_easy / num_valid

Sharding: samples sorted by class; core r owns 80 consecutive anchors of the
sorted order. Each core gets its OWN sample layout: the T classes its
anchors span sit in 128-aligned "class tiles" (padded with huge-norm dummy
embeddings, label -1), followed by the remaining samples padded to a common
Nk. Every anchor's positives then occupy ONE class tile, so the per-anchor
comparison matrix M[p, k] = (v'_k < t'_p) is a single [128, Nk] bf16 tile.
Pads can never be positives (label -1) nor hard negatives (distance ~3e4 >>
any threshold), and loss sums/counts are sample-order invariant.

Per anchor a:
  - masked row v'_k = d_ak + BIG*(same label), bf16, staged to DRAM and
    DMA-broadcast to [128, 8*Nk] for 8 anchors at a time.
  - thresholds t'_p = (d(a, member p of a's class) + margin) * posC, built
    from T transposed DIST tiles muxed by per-anchor select masks, hi/lo
    split into bf16.
  - M built in ONE bf16 op: DVE tensor_scalar is_lt for 5 of 8 anchors;
    ACT Sign (corrected on host) for 3 of 8.
  - PE reduces M with a zero-padded [128, 32] lhsT (layout [128, col, a]):
    anchor slot s=a%8 has (t_hi, t_lo) at cols (2s, 2s+1) and 1.0 at col
    16+s, so 8 anchors share one PE output quadrant and 24 anchors share a
    psum bank pair; 2 matmuls per anchor (k split 512 + Nk-512).
  - drain (4x per core): ACT free-sums psum rows straight into the output
    tile; DVE fused (q * dist) reduce likewise.
  total = sum(t'*M) - sum(d*q);  count = sum(q).  Host combines in f64.
num_valid is pure label counting (host, exact).
"""

import numpy as np
import ml_dtypes

N = 640
D = 128
NCORES = 8
NLOC = N // NCORES            # 80 anchors per core
GRPA = 24                     # anchors per psum group (3 quadrants x 8)
NGRP = (NLOC + GRPA - 1) // GRPA   # 4 groups: 24, 24, 24, 8
VBB = 8                       # anchors per v'-broadcast DMA
MARGIN = 1.9
BIG = 1.0e9
PADV = 3.0e4                  # pad embedding magnitude


def _is_sign_anchor(a: int) -> bool:
    return a % 8 in (0, 3, 6)


_CACHE = {}


def _build_program(t_cls: int, nk: int, r_rows: int):
    import concourse.bass as bass
    import concourse.bacc as bacc
    import concourse.mybir as mybir
    import concourse.tile as tile
    from concourse.masks import make_identity

    f32 = mybir.dt.float32
    bf16 = mybir.dt.bfloat16
    Alu = mybir.AluOpType
    Act = mybir.ActivationFunctionType

    NK = nk
    NK2 = NK - 512            # second psum bank width
    R = r_rows                # M-matrix rows = max class size

    nc = bacc.Bacc("TRN2", target_bir_lowering=False, debug=False,
                   num_devices=NCORES)

    efT = nc.declare_dram_parameter("efT", [D, NK], f32, isOutput=False)
    elocT = nc.declare_dram_parameter("elocT", [D, NLOC], f32, isOutput=False)
    labrow = nc.declare_dram_parameter("labrow", [1, NK], f32, isOutput=False)
    llocT = nc.declare_dram_parameter("llocT", [NLOC, 1], f32, isOutput=False)
    posc = nc.declare_dram_parameter("posc", [128, NLOC], f32, isOutput=False)
    selb = nc.declare_dram_parameter("selb", [128, t_cls * NLOC], f32,
                                     isOutput=False)
    # out: [128, 2*NGRP(wsums) + 2*NGRP(p2) + 1 (dist row sums)]
    out_d = nc.declare_dram_parameter("out", [128, 4 * NGRP + 1], f32,
                                      isOutput=True)
    # out2: [1, NLOC] Tsum_a = sum_p t'_ap (for ACT-sign anchors' corrections)
    out2_d = nc.declare_dram_parameter("out2", [1, NLOC], f32, isOutput=True)

    from contextlib import ExitStack
    with tile.TileContext(nc) as tc:
        with (
            tc.tile_pool(name="singles", bufs=1) as sg,
            tc.tile_pool(name="vbp", bufs=4) as vbp,
            tc.tile_pool(name="mtp", bufs=16) as mtp,
            tc.tile_pool(name="dpp", bufs=2) as dpp,
            tc.tile_pool(name="drs", bufs=3) as drs,
            tc.tile_pool(name="dram", bufs=1, space="DRAM") as dram,
        ):
            pro_stack = ExitStack()
            ps_mm = pro_stack.enter_context(
                tc.tile_pool(name="ps_mm", bufs=1, space="PSUM"))
            ps_tr = pro_stack.enter_context(
                tc.tile_pool(name="ps_tr", bufs=1, space="PSUM"))
            # ---- load inputs ----
            EF = sg.tile([D, NK], f32)
            nc.sync.dma_start(out=EF[:], in_=efT[:])
            EL = sg.tile([D, NLOC], f32)
            nc.sync.dma_start(out=EL[:], in_=elocT[:])
            LLT = sg.tile([NLOC, 1], f32)
            nc.gpsimd.dma_start(out=LLT[:], in_=llocT[:])
            POSC = sg.tile([128, NLOC], f32)
            nc.gpsimd.dma_start(out=POSC[:], in_=posc[:])
            SELB = sg.tile([128, t_cls * NLOC], f32)
            nc.gpsimd.dma_start(out=SELB[:], in_=selb[:])
            # zero-padded quadrant lhsT tile, zeroed + ones pre-filled early
            row_d = 32 * NLOC
            L32 = sg.tile([128, 32, NLOC], bf16, name="l32")
            nc.gpsimd.memset(L32[:], 0.0)
            for s in range(8):
                dst_o = bass.AP(tensor=L32[:].tensor,
                                offset=L32[:].offset + (16 + s) * NLOC + s,
                                ap=[[row_d, 128], [8, NLOC // 8]])
                nc.gpsimd.memset(dst_o, 1.0)

            ident = sg.tile([128, 128], f32)
            make_identity(nc, ident[:])
            ones = sg.tile([128, 1], f32)
            nc.vector.memset(ones[:], 1.0)

            # ---- pairwise distance rows for local anchors ----
            Esq = sg.tile([D, NK], f32)
            nc.vector.tensor_mul(Esq[:], EF[:], EF[:])
            ELsq = sg.tile([D, NLOC], f32)
            nc.vector.tensor_mul(ELsq[:], EL[:], EL[:])

            sqf1 = ps_mm.tile([1, 512], f32, tag="s1", name="sqf1")
            nc.tensor.matmul(sqf1[:], ones[:], Esq[:, 0:512])
            sqf2 = ps_mm.tile([1, NK2], f32, tag="s2", name="sqf2")
            nc.tensor.matmul(sqf2[:], ones[:], Esq[:, 512:NK])
            SQF = sg.tile([1, NK], f32)
            nc.vector.tensor_copy(SQF[:, 0:512], sqf1[:])
            nc.vector.tensor_copy(SQF[:, 512:NK], sqf2[:])

            sql_ps = ps_mm.tile([NLOC, 1], f32, tag="sq", name="sql")
            nc.tensor.matmul(sql_ps[:], ELsq[:], ones[:])
            SQL = sg.tile([NLOC, 1], f32)
            nc.vector.tensor_copy(SQL[:], sql_ps[:])

            dot1 = ps_mm.tile([NLOC, 512], f32, tag="d1", name="dot1")
            nc.tensor.matmul(dot1[:], EL[:], EF[:, 0:512])
            dot2 = ps_mm.tile([NLOC, NK2], f32, tag="d2", name="dot2")
            nc.tensor.matmul(dot2[:], EL[:], EF[:, 512:NK])

            A = sg.tile([NLOC, NK], f32)
            nc.vector.tensor_scalar(out=A[:, 0:512], in0=dot1[:], scalar1=-2.0,
                                    scalar2=SQL[:], op0=Alu.mult, op1=Alu.add)
            nc.vector.tensor_scalar(out=A[:, 512:NK], in0=dot2[:], scalar1=-2.0,
                                    scalar2=SQL[:], op0=Alu.mult, op1=Alu.add)
            # broadcast sq_k to all anchor rows via PE (1-row contraction)
            ones1 = sg.tile([1, NLOC], f32)
            nc.vector.memset(ones1[:], 1.0)
            sqb1 = ps_tr.tile([NLOC, 512], f32, tag="tr1", name="sqb1")
            nc.tensor.matmul(sqb1[:], ones1[:], SQF[:, 0:512])
            sqb2 = ps_tr.tile([NLOC, NK2], f32, tag="tr2", name="sqb2")
            nc.tensor.matmul(sqb2[:], ones1[:], SQF[:, 512:NK])
            PRE = sg.tile([NLOC, NK], f32)
            nc.vector.tensor_add(PRE[:, 0:512], A[:, 0:512], sqb1[:])
            nc.vector.tensor_add(PRE[:, 512:NK], A[:, 512:NK], sqb2[:])
            nc.vector.tensor_scalar(out=PRE[:], in0=PRE[:], scalar1=0.0,
                                    scalar2=None, op0=Alu.max)
            DIST = sg.tile([NLOC, NK], f32)
            nc.scalar.activation(out=DIST[:], in_=PRE[:], func=Act.Sqrt)

            # masked v' row, bf16
            LBC = sg.tile([128, NK], f32)
            nc.sync.dma_start(out=LBC[0:NLOC, :],
                              in_=labrow[:].to_broadcast([NLOC, NK]))
            EQB = sg.tile([NLOC, NK], f32)
            nc.vector.tensor_scalar(out=EQB[:], in0=LBC[0:NLOC, :], scalar1=LLT[:],
                                    scalar2=BIG, op0=Alu.is_equal, op1=Alu.mult)
            VM = sg.tile([NLOC, NK], f32)
            VMB = sg.tile([NLOC, NK], bf16)
            vmd = dram.tile([NLOC, NK], bf16)
            # fast-path the first 32 anchors so vb batch 0 launches early
            nc.vector.tensor_add(VM[0:32, :], DIST[0:32, :], EQB[0:32, :])
            nc.vector.tensor_copy(VMB[0:32, :], VM[0:32, :])
            nc.sync.dma_start(out=vmd[0:32, :], in_=VMB[0:32, :])
            nc.vector.tensor_add(VM[32:64, :], DIST[32:64, :], EQB[32:64, :])
            nc.vector.tensor_copy(VMB[32:64, :], VM[32:64, :])
            nc.sync.dma_start(out=vmd[32:64, :], in_=VMB[32:64, :])
            nc.vector.tensor_add(VM[64:NLOC, :], DIST[64:NLOC, :],
                                 EQB[64:NLOC, :])
            nc.vector.tensor_copy(VMB[64:NLOC, :], VM[64:NLOC, :])
            nc.sync.dma_start(out=vmd[64:NLOC, :], in_=VMB[64:NLOC, :])

            # thresholds: mux T transposed DIST class tiles by select masks,
            # then t' = (TPC + margin) * posC, hi/lo split to bf16.
            trs = []
            for c in range(t_cls):
                tr_ps = ps_tr.tile([128, NLOC], f32, tag=f"tr{c}",
                                   name=f"tr{c}")
                nc.tensor.transpose(tr_ps[:], DIST[:, c * 128:(c + 1) * 128],
                                    ident[0:NLOC, 0:NLOC])
                trs.append(tr_ps)
            TPC = sg.tile([128, NLOC], f32)
            nc.vector.tensor_mul(TPC[:], trs[0][:], SELB[:, 0:NLOC])
            for c in range(1, t_cls):
                PP = sg.tile([128, NLOC], f32, tag=f"pp{c}", name=f"pp{c}")
                nc.vector.tensor_mul(PP[:], trs[c][:],
                                     SELB[:, c * NLOC:(c + 1) * NLOC])
                nc.vector.tensor_add(TPC[:], TPC[:], PP[:])
            tp = sg.tile([128, NLOC], f32, name="tp")
            nc.vector.tensor_scalar_add(out=tp[:], in0=TPC[:], scalar1=MARGIN)
            nc.vector.tensor_mul(tp[:], tp[:], POSC[:])

            TH = sg.tile([128, NLOC], bf16, name="thb")
            nc.vector.tensor_copy(TH[:], tp[:])                # t_hi (bf16)
            thf = sg.tile([128, NLOC], f32, name="thf")
            nc.vector.tensor_copy(thf[:], TH[:])               # back to f32
            nc.vector.tensor_sub(thf[:], tp[:], thf[:])        # t_lo
            TL = sg.tile([128, NLOC], bf16, name="tlb")
            nc.vector.tensor_copy(TL[:], thf[:])
            # fill zero-padded quadrant lhsT (layout [128, col, a]):
            # anchor slot s=a%8 has (hi, lo) at cols (2s, 2s+1); ones
            # pre-filled at col 16+s. matmul reads col-strided.
            row_s = NLOC
            for s in range(8):
                nhere = NLOC // 8
                src_h = bass.AP(tensor=TH[:].tensor, offset=TH[:].offset + s,
                                ap=[[row_s, 128], [8, nhere]])
                dst_h = bass.AP(tensor=L32[:].tensor,
                                offset=L32[:].offset + 2 * s * NLOC + s,
                                ap=[[row_d, 128], [8, nhere]])
                nc.vector.tensor_copy(dst_h, src_h)
                src_l = bass.AP(tensor=TL[:].tensor, offset=TL[:].offset + s,
                                ap=[[row_s, 128], [8, nhere]])
                dst_l = bass.AP(tensor=L32[:].tensor,
                                offset=L32[:].offset + (2 * s + 1) * NLOC + s,
                                ap=[[row_d, 128], [8, nhere]])
                nc.gpsimd.tensor_copy(dst_l, src_l)

            # dist row sums (for sign-anchor corrections) -> OUTS directly
            OUTS = sg.tile([128, 4 * NGRP + 1], f32)
            DSC = sg.tile([NLOC, NK], f32)
            nc.scalar.activation(out=DSC[:], in_=DIST[:], func=Act.Identity,
                                 bias=0.0, scale=1.0,
                                 accum_out=OUTS[0:NLOC, 4 * NGRP:4 * NGRP + 1])

            # Tsum_a = sum_p t'_ap
            ts_ps = ps_tr.tile([1, NLOC], f32, tag="tr0", name="ts_ps")
            nc.tensor.matmul(ts_ps[:], ones[:], tp[:])
            TSROW = sg.tile([1, NLOC], f32)
            nc.vector.tensor_copy(TSROW[:], ts_ps[:])
            nc.sync.dma_start(out=out2_d[:], in_=TSROW[:])

            pro_stack.close()
            wq_stack = ExitStack()
            ps_wq1 = wq_stack.enter_context(
                tc.tile_pool(name="ps_wq1", bufs=2, space="PSUM"))
            ps_wq2 = wq_stack.enter_context(
                tc.tile_pool(name="ps_wq2", bufs=2, space="PSUM"))

            # ---- main loop ----
            vb_cache = {}
            for g in range(NGRP):
                na = min(GRPA, NLOC - GRPA * g)
                nqd = (na + 7) // 8
                wq1 = ps_wq1.tile([128, 512], f32, tag="wq1", name="wq1")
                wq2 = ps_wq2.tile([128, NK2], f32, tag="wq2", name="wq2")
                dp = dpp.tile([128, NK], f32, tag="dp", name="dp")
                # dist rows of quadrant's anchors -> dp partitions 32*qd+16+s
                for qd in range(nqd):
                    bn = min(8, na - 8 * qd)
                    a0 = GRPA * g + 8 * qd
                    nc.gpsimd.dma_start(
                        out=dp[32 * qd + 16:32 * qd + 16 + bn, :],
                        in_=DIST[a0:a0 + bn, :])
                for m in range(na):
                    a = GRPA * g + m
                    qd, s8 = m // 8, m % 8
                    bn = min(8, na - 8 * qd)
                    if a % VBB == 0:
                        vb2 = vbp.tile([R, VBB, NK], bf16, tag="vb",
                                       name="vb")
                        sl = vmd[a:a + VBB, :]
                        bsrc = bass.AP(tensor=sl.tensor, offset=sl.offset,
                                       ap=[[0, R]] + [list(q) for q in sl.ap])
                        nc.sync.dma_start(out=vb2[:], in_=bsrc)
                        vb_cache[0] = vb2
                    vb = vb_cache[0][:, a % VBB, :]
                    on_act = _is_sign_anchor(a)
                    st = (s8 == 0)
                    sp = (s8 == bn - 1)
                    mt = mtp.tile([R, NK], bf16, tag="mt", name="mt")
                    if on_act:
                        nc.scalar.activation(out=mt[:], in_=vb[:],
                                             func=Act.Sign,
                                             bias=tp[0:R, a:a + 1],
                                             scale=-1.0)
                    else:
                        nc.vector.tensor_scalar(out=mt[:], in0=vb[:],
                                                scalar1=tp[0:R, a:a + 1],
                                                scalar2=None, op0=Alu.is_lt)
                    nc.tensor.matmul(wq1[32 * qd:32 * qd + 32, :],
                                     L32[0:R, :, a], mt[:, 0:512],
                                     start=st, stop=sp)
                    nc.tensor.matmul(wq2[32 * qd:32 * qd + 32, :],
                                     L32[0:R, :, a], mt[:, 512:NK],
                                     start=st, stop=sp)
                # drain group: ACT free-sums psum rows; DVE fused q*dist;
                # both accumulate straight into OUTS columns.
                sa1 = drs.tile([128, 512], f32, tag="sa1", name="sa1")
                sa2 = drs.tile([128, NK2], f32, tag="sa2", name="sa2")
                sb1 = drs.tile([128, 512], f32, tag="sb1", name="sb1")
                sb2 = drs.tile([128, NK2], f32, tag="sb2", name="sb2")
                nc.scalar.activation(out=sa1[:], in_=wq1[:], func=Act.Identity,
                                     bias=0.0, scale=1.0,
                                     accum_out=OUTS[:, 2 * g:2 * g + 1])
                nc.scalar.activation(out=sa2[:], in_=wq2[:], func=Act.Identity,
                                     bias=0.0, scale=1.0,
                                     accum_out=OUTS[:, 2 * g + 1:2 * g + 2])
                nc.vector.scalar_tensor_tensor(out=sb1[:], in0=wq1[:],
                                               scalar=1.0, in1=dp[:, 0:512],
                                               op0=Alu.mult, op1=Alu.mult,
                                               accum_out=OUTS[:, 2 * NGRP + 2 * g:
                                                              2 * NGRP + 2 * g + 1])
                nc.vector.scalar_tensor_tensor(out=sb2[:], in0=wq2[:],
                                               scalar=1.0, in1=dp[:, 512:NK],
                                               op0=Alu.mult, op1=Alu.mult,
                                               accum_out=OUTS[:, 2 * NGRP + 2 * g + 1:
                                                              2 * NGRP + 2 * g + 2])

            nc.gpsimd.dma_start(out=out_d[:], in_=OUTS[:])
            wq_stack.close()

    nc.compile()
    return nc


def _get_program(t_cls: int, nk: int, r_rows: int):
    key = ("nc", t_cls, nk, r_rows)
    if key not in _CACHE:
        _CACHE[key] = _build_program(t_cls, nk, r_rows)
    return _CACHE[key]


def _plan_layout(lab: np.ndarray):
    """Class-sort the samples; per core: the T classes its anchors span go
    into 128-aligned class tiles whose slack slots are filled with samples
    of other (non-spanned) classes, then the rest. Exactly N columns."""
    order = np.argsort(lab, kind="stable")
    slab = lab[order]
    spans = []
    t_cls = 0
    for r in range(NCORES):
        lo = slab[NLOC * r]
        hi = slab[NLOC * r + NLOC - 1]
        cls = []
        for c in range(int(lo), int(hi) + 1):
            i0 = int(np.searchsorted(slab, c, "left"))
            i1 = int(np.searchsorted(slab, c, "right"))
            if i1 > i0:
                cls.append((c, i0, i1))
                assert i1 - i0 <= 128, "class larger than 128"
        spans.append(cls)
        t_cls = max(t_cls, len(cls))
    r_rows = max(i1 - i0 for cls in spans for _, i0, i1 in cls)
    nk = N
    assert 128 * t_cls <= N, "class tiles exceed sample count"

    plans = []
    for r in range(NCORES):
        cls = spans[r]
        in_span = np.zeros(N, bool)
        pos = -np.ones(N, np.int64)      # sorted-idx -> column in core layout
        for t, (c, i0, i1) in enumerate(cls):
            in_span[i0:i1] = True
            pos[i0:i1] = 128 * t + np.arange(i1 - i0)
        rest = list(np.where(~in_span)[0])
        # fill class-tile slack with non-spanned samples (true negatives)
        for t, (c, i0, i1) in enumerate(cls):
            for p in range(i1 - i0, 128):
                pos[rest.pop()] = 128 * t + p
        nxt = 128 * len(cls)
        for sidx in rest:
            pos[sidx] = nxt
            nxt += 1
        assert nxt == N
        anchors = np.arange(NLOC * r, NLOC * r + NLOC)
        a_tile = np.zeros(NLOC, np.int64)
        a_q = np.zeros(NLOC, np.int64)
        for i, asort in enumerate(anchors):
            hit = False
            for t, (c, i0, i1) in enumerate(cls):
                if i0 <= asort < i1:
                    a_tile[i] = t
                    a_q[i] = asort - i0
                    hit = True
                    break
            assert hit, "anchor not inside its span"
        n_per_tile = [i1 - i0 for _, i0, i1 in cls]
        plans.append((pos, a_tile, a_q, n_per_tile))
    return plans, order, t_cls, nk, r_rows


def _make_inputs(embeddings: np.ndarray, labels: np.ndarray):
    e = np.ascontiguousarray(embeddings.reshape(N, D).astype(np.float32))
    lab = labels.reshape(N).astype(np.float32)
    plans, order, t_cls, nk, r_rows = _plan_layout(lab)

    in_maps = []
    for r in range(NCORES):
        pos, a_tile, a_q, n_per_tile = plans[r]
        ef = np.zeros((nk, D), np.float32)
        labr = np.zeros(nk, np.float32)
        ef[pos] = e[order]
        labr[pos] = lab[order]
        efTr = np.ascontiguousarray(ef.T)                # [D, nk]
        apos = 128 * a_tile + a_q                        # anchor columns
        poscm = np.zeros((128, NLOC), np.float32)
        for i in range(NLOC):
            nt = n_per_tile[a_tile[i]]
            poscm[:nt, i] = 1.0
            poscm[a_q[i], i] = 0.0
        selbm = np.zeros((128, t_cls * NLOC), np.float32)
        for i in range(NLOC):
            selbm[:, a_tile[i] * NLOC + i] = 1.0
        in_maps.append({
            "efT": efTr,
            "elocT": np.ascontiguousarray(efTr[:, apos]),
            "labrow": labr.reshape(1, nk),
            "llocT": np.ascontiguousarray(labr[apos].reshape(NLOC, 1)),
            "posc": poscm,
            "selb": selbm,
        })
    return in_maps, t_cls, nk, r_rows


def run_on_device(embeddings: np.ndarray, labels: np.ndarray, **run_kwargs):
    from concourse.bass_utils import run_bass_kernel_spmd
    in_maps, t_cls, nk, r_rows = _make_inputs(embeddings, labels)
    nc = _get_program(t_cls, nk, r_rows)
    res = run_bass_kernel_spmd(nc, in_maps, core_ids=list(range(NCORES)),
                               **run_kwargs)
    total = 0.0
    count = 0.0
    for r in range(NCORES):
        o = res.results[r]["out"].astype(np.float64)
        tsum = res.results[r]["out2"].astype(np.float64).reshape(-1)
        dsum = o[0:NLOC, 4 * NGRP]
        for g in range(NGRP):
            na = min(GRPA, NLOC - GRPA * g)
            for m in range(na):
                a = GRPA * g + m
                qd, s8 = m // 8, m % 8
                bw = 32 * qd + 2 * s8
                bq = 32 * qd + 16 + s8
                w = q = p2 = 0.0
                for ch in range(2):
                    w += o[bw + 0, 2 * g + ch] + o[bw + 1, 2 * g + ch]
                    q += o[bq, 2 * g + ch]
                    p2 += o[bq, 2 * NGRP + 2 * g + ch]
                if _is_sign_anchor(a):
                    w = 0.5 * w + 0.5 * nk * tsum[a]
                    q = 0.5 * q + 0.5 * r_rows * nk
                    p2 = 0.5 * p2 + 0.5 * r_rows * dsum[a]
                total += w - p2
                count += q
    return total, count, res


def kernel(embeddings: np.ndarray, labels: np.ndarray):
    embeddings = np.asarray(embeddings)
    labels = np.asarray(labels)
    total, count, _ = run_on_device(embeddings, labels)

    lab = np.asarray(labels).reshape(-1)
    cnt = np.bincount(lab.astype(np.int64), minlength=1)
    per = cnt[lab.astype(np.int64)]
    num_valid = int(((per - 1) * (N - per)).sum())

    nv = np.float32(num_valid)
    ne = np.float32(count)
    tot = np.float32(total)
    if ne > 0:
        loss = np.float32(tot / np.maximum(ne, np.float32(1.0)))
    else:
        loss = np.float32(0.0)
    frac = np.float32(ne / (nv + np.float32(1e-16)))
    return (np.array(loss, np.float32), np.array(nv, np.float32),
            np.array(ne, np.float32), np.array(frac, np.float32))


# revision 36
# speedup vs baseline: 1.0128x; 1.0128x over previous
"""BatchAllTripletLoss on 8 Trainium2 NeuronCores via Bass/Tile.

Math: for anchors i, positives j (same label, j!=i), negatives k (diff label):
  total        = sum_{i,j,k} relu(d_ij - d_ik + margin)
  num_non_easy = #{(i,j,k): d_ik < d_ij + margin}
  loss         = total / num_non_easy ; frac = num_non_easy / num_valid

Sharding: samples sorted by class; core r owns 80 consecutive anchors of the
sorted order. Each core gets its OWN sample layout: the T classes its
anchors span sit in 128-aligned "class tiles" (padded with huge-norm dummy
embeddings, label -1), followed by the remaining samples padded to a common
Nk. Every anchor's positives then occupy ONE class tile, so the per-anchor
comparison matrix M[p, k] = (v'_k < t'_p) is a single [128, Nk] bf16 tile.
Pads can never be positives (label -1) nor hard negatives (distance ~3e4 >>
any threshold), and loss sums/counts are sample-order invariant.

Per anchor a:
  - masked row v'_k = d_ak + BIG*(same label), bf16, staged to DRAM and
    DMA-broadcast to [128, 8*Nk] for 8 anchors at a time.
  - thresholds t'_p = (d(a, member p of a's class) + margin) * posC, built
    from T transposed DIST tiles muxed by per-anchor select masks, hi/lo
    split into bf16.
  - M built in ONE bf16 op: DVE tensor_scalar is_lt for 5 of 8 anchors;
    ACT Sign (corrected on host) for 3 of 8.
  - PE reduces M with a zero-padded [128, 32] lhsT (layout [128, col, a]):
    anchor slot s=a%8 has (t_hi, t_lo) at cols (2s, 2s+1) and 1.0 at col
    16+s, so 8 anchors share one PE output quadrant and 24 anchors share a
    psum bank pair; 2 matmuls per anchor (k split 512 + Nk-512).
  - drain (4x per core): ACT free-sums psum rows straight into the output
    tile; DVE fused (q * dist) reduce likewise.
  total = sum(t'*M) - sum(d*q);  count = sum(q).  Host combines in f64.
num_valid is pure label counting (host, exact).
"""

import numpy as np

N = 640
D = 128
NCORES = 8
NLOC = N // NCORES            # 80 anchors per core
GRPA = 24                     # anchors per psum group (3 quadrants x 8)
NGRP = (NLOC + GRPA - 1) // GRPA   # 4 groups: 24, 24, 24, 8
VBB = 8                       # anchors per v'-broadcast DMA
MARGIN = 1.9
BIG = 1.0e9
PADV = 3.0e4                  # pad embedding magnitude


def _is_sign_anchor(a: int) -> bool:
    return a % 8 in (0, 3, 6)


_CACHE = {}


def _build_program(t_cls: int, nk: int, r_rows: int):
    import concourse.bass as bass
    import concourse.bacc as bacc
    import concourse.mybir as mybir
    import concourse.tile as tile
    from concourse.masks import make_identity

    f32 = mybir.dt.float32
    bf16 = mybir.dt.bfloat16
    Alu = mybir.AluOpType
    Act = mybir.ActivationFunctionType

    NK = nk
    NK2 = NK - 512            # second psum bank width
    R = r_rows                # M-matrix rows = max class size

    nc = bacc.Bacc("TRN2", target_bir_lowering=False, debug=False,
                   num_devices=NCORES)

    efT = nc.declare_dram_parameter("efT", [D, NK], f32, isOutput=False)
    elocT = nc.declare_dram_parameter("elocT", [D, NLOC], f32, isOutput=False)
    labrow = nc.declare_dram_parameter("labrow", [1, NK], f32, isOutput=False)
    llocT = nc.declare_dram_parameter("llocT", [NLOC, 1], f32, isOutput=False)
    posc = nc.declare_dram_parameter("posc", [128, NLOC], f32, isOutput=False)
    selb = nc.declare_dram_parameter("selb", [128, t_cls * NLOC], f32,
                                     isOutput=False)
    # out: [128, 2*NGRP(wsums) + 2*NGRP(p2) + 1 (dist row sums)]
    out_d = nc.declare_dram_parameter("out", [128, 4 * NGRP + 1], f32,
                                      isOutput=True)
    # out2: [1, NLOC] Tsum_a = sum_p t'_ap (for ACT-sign anchors' corrections)
    out2_d = nc.declare_dram_parameter("out2", [1, NLOC], f32, isOutput=True)

    from contextlib import ExitStack
    with tile.TileContext(nc) as tc:
        with (
            tc.tile_pool(name="singles", bufs=1) as sg,
            tc.tile_pool(name="vbp", bufs=4) as vbp,
            tc.tile_pool(name="mtp", bufs=16) as mtp,
            tc.tile_pool(name="dpp", bufs=2) as dpp,
            tc.tile_pool(name="drs", bufs=3) as drs,
            tc.tile_pool(name="dram", bufs=1, space="DRAM") as dram,
        ):
            pro_stack = ExitStack()
            ps_mm = pro_stack.enter_context(
                tc.tile_pool(name="ps_mm", bufs=1, space="PSUM"))
            ps_tr = pro_stack.enter_context(
                tc.tile_pool(name="ps_tr", bufs=1, space="PSUM"))
            # ---- load inputs ----
            EF = sg.tile([D, NK], f32)
            nc.sync.dma_start(out=EF[:], in_=efT[:])
            EL = sg.tile([D, NLOC], f32)
            nc.sync.dma_start(out=EL[:], in_=elocT[:])
            LLT = sg.tile([NLOC, 1], f32)
            nc.gpsimd.dma_start(out=LLT[:], in_=llocT[:])
            POSC = sg.tile([128, NLOC], f32)
            nc.gpsimd.dma_start(out=POSC[:], in_=posc[:])
            SELB = sg.tile([128, t_cls * NLOC], f32)
            nc.gpsimd.dma_start(out=SELB[:], in_=selb[:])
            # zero-padded quadrant lhsT tile, zeroed + ones pre-filled early
            row_d = 32 * NLOC
            L32 = sg.tile([128, 32, NLOC], bf16, name="l32")
            nc.gpsimd.memset(L32[:], 0.0)
            for s in range(8):
                dst_o = bass.AP(tensor=L32[:].tensor,
                                offset=L32[:].offset + (16 + s) * NLOC + s,
                                ap=[[row_d, 128], [8, NLOC // 8]])
                nc.gpsimd.memset(dst_o, 1.0)

            ident = sg.tile([128, 128], f32)
            make_identity(nc, ident[:])
            ones = sg.tile([128, 1], f32)
            nc.vector.memset(ones[:], 1.0)

            # ---- pairwise distance rows for local anchors ----
            Esq = sg.tile([D, NK], f32)
            nc.vector.tensor_mul(Esq[:], EF[:], EF[:])
            ELsq = sg.tile([D, NLOC], f32)
            nc.vector.tensor_mul(ELsq[:], EL[:], EL[:])

            sqf1 = ps_mm.tile([1, 512], f32, tag="s1", name="sqf1")
            nc.tensor.matmul(sqf1[:], ones[:], Esq[:, 0:512])
            sqf2 = ps_mm.tile([1, NK2], f32, tag="s2", name="sqf2")
            nc.tensor.matmul(sqf2[:], ones[:], Esq[:, 512:NK])
            SQF = sg.tile([1, NK], f32)
            nc.vector.tensor_copy(SQF[:, 0:512], sqf1[:])
            nc.vector.tensor_copy(SQF[:, 512:NK], sqf2[:])

            sql_ps = ps_mm.tile([NLOC, 1], f32, tag="sq", name="sql")
            nc.tensor.matmul(sql_ps[:], ELsq[:], ones[:])
            SQL = sg.tile([NLOC, 1], f32)
            nc.vector.tensor_copy(SQL[:], sql_ps[:])

            dot1 = ps_mm.tile([NLOC, 512], f32, tag="d1", name="dot1")
            nc.tensor.matmul(dot1[:], EL[:], EF[:, 0:512])
            dot2 = ps_mm.tile([NLOC, NK2], f32, tag="d2", name="dot2")
            nc.tensor.matmul(dot2[:], EL[:], EF[:, 512:NK])

            A = sg.tile([NLOC, NK], f32)
            nc.vector.tensor_scalar(out=A[:, 0:512], in0=dot1[:], scalar1=-2.0,
                                    scalar2=SQL[:], op0=Alu.mult, op1=Alu.add)
            nc.vector.tensor_scalar(out=A[:, 512:NK], in0=dot2[:], scalar1=-2.0,
                                    scalar2=SQL[:], op0=Alu.mult, op1=Alu.add)
            # broadcast sq_k to all anchor rows via PE (1-row contraction)
            ones1 = sg.tile([1, NLOC], f32)
            nc.vector.memset(ones1[:], 1.0)
            sqb1 = ps_tr.tile([NLOC, 512], f32, tag="tr1", name="sqb1")
            nc.tensor.matmul(sqb1[:], ones1[:], SQF[:, 0:512])
            sqb2 = ps_tr.tile([NLOC, NK2], f32, tag="tr2", name="sqb2")
            nc.tensor.matmul(sqb2[:], ones1[:], SQF[:, 512:NK])
            PRE = sg.tile([NLOC, NK], f32)
            nc.vector.tensor_add(PRE[:, 0:512], A[:, 0:512], sqb1[:])
            nc.vector.tensor_add(PRE[:, 512:NK], A[:, 512:NK], sqb2[:])
            nc.vector.tensor_scalar(out=PRE[:], in0=PRE[:], scalar1=0.0,
                                    scalar2=None, op0=Alu.max)
            DIST = sg.tile([NLOC, NK], f32)
            nc.scalar.activation(out=DIST[:], in_=PRE[:], func=Act.Sqrt)

            # masked v' row, bf16
            LBC = sg.tile([128, NK], f32)
            nc.sync.dma_start(out=LBC[0:NLOC, :],
                              in_=labrow[:].to_broadcast([NLOC, NK]))
            EQB = sg.tile([NLOC, NK], f32)
            nc.vector.tensor_scalar(out=EQB[:], in0=LBC[0:NLOC, :], scalar1=LLT[:],
                                    scalar2=BIG, op0=Alu.is_equal, op1=Alu.mult)
            VM = sg.tile([NLOC, NK], f32)
            VMB = sg.tile([NLOC, NK], bf16)
            vmd = dram.tile([NLOC, NK], bf16)
            # fast-path the first 32 anchors so vb batch 0 launches early
            nc.vector.tensor_add(VM[0:32, :], DIST[0:32, :], EQB[0:32, :])
            nc.vector.tensor_copy(VMB[0:32, :], VM[0:32, :])
            nc.sync.dma_start(out=vmd[0:32, :], in_=VMB[0:32, :])
            nc.vector.tensor_add(VM[32:64, :], DIST[32:64, :], EQB[32:64, :])
            nc.vector.tensor_copy(VMB[32:64, :], VM[32:64, :])
            nc.sync.dma_start(out=vmd[32:64, :], in_=VMB[32:64, :])
            nc.vector.tensor_add(VM[64:NLOC, :], DIST[64:NLOC, :],
                                 EQB[64:NLOC, :])
            nc.vector.tensor_copy(VMB[64:NLOC, :], VM[64:NLOC, :])
            nc.sync.dma_start(out=vmd[64:NLOC, :], in_=VMB[64:NLOC, :])

            # thresholds: mux T transposed DIST class tiles by select masks,
            # then t' = (TPC + margin) * posC, hi/lo split to bf16.
            trs = []
            for c in range(t_cls):
                tr_ps = ps_tr.tile([128, NLOC], f32, tag=f"tr{c}",
                                   name=f"tr{c}")
                nc.tensor.transpose(tr_ps[:], DIST[:, c * 128:(c + 1) * 128],
                                    ident[0:NLOC, 0:NLOC])
                trs.append(tr_ps)
            TPC = sg.tile([128, NLOC], f32)
            nc.vector.tensor_mul(TPC[:], trs[0][:], SELB[:, 0:NLOC])
            for c in range(1, t_cls):
                PP = sg.tile([128, NLOC], f32, tag=f"pp{c}", name=f"pp{c}")
                nc.vector.tensor_mul(PP[:], trs[c][:],
                                     SELB[:, c * NLOC:(c + 1) * NLOC])
                nc.vector.tensor_add(TPC[:], TPC[:], PP[:])
            tp = sg.tile([128, NLOC], f32, name="tp")
            nc.vector.tensor_scalar_add(out=tp[:], in0=TPC[:], scalar1=MARGIN)
            nc.vector.tensor_mul(tp[:], tp[:], POSC[:])

            TH = sg.tile([128, NLOC], bf16, name="thb")
            nc.vector.tensor_copy(TH[:], tp[:])                # t_hi (bf16)
            thf = sg.tile([128, NLOC], f32, name="thf")
            nc.vector.tensor_copy(thf[:], TH[:])               # back to f32
            nc.vector.tensor_sub(thf[:], tp[:], thf[:])        # t_lo
            TL = sg.tile([128, NLOC], bf16, name="tlb")
            nc.vector.tensor_copy(TL[:], thf[:])
            # fill zero-padded quadrant lhsT (layout [128, col, a]):
            # anchor slot s=a%8 has (hi, lo) at cols (2s, 2s+1); ones
            # pre-filled at col 16+s. matmul reads col-strided.
            row_s = NLOC
            for s in range(8):
                nhere = NLOC // 8
                src_h = bass.AP(tensor=TH[:].tensor, offset=TH[:].offset + s,
                                ap=[[row_s, 128], [8, nhere]])
                dst_h = bass.AP(tensor=L32[:].tensor,
                                offset=L32[:].offset + 2 * s * NLOC + s,
                                ap=[[row_d, 128], [8, nhere]])
                nc.vector.tensor_copy(dst_h, src_h)
                src_l = bass.AP(tensor=TL[:].tensor, offset=TL[:].offset + s,
                                ap=[[row_s, 128], [8, nhere]])
                dst_l = bass.AP(tensor=L32[:].tensor,
                                offset=L32[:].offset + (2 * s + 1) * NLOC + s,
                                ap=[[row_d, 128], [8, nhere]])
                nc.gpsimd.tensor_copy(dst_l, src_l)

            # dist row sums (for sign-anchor corrections) -> OUTS directly
            OUTS = sg.tile([128, 4 * NGRP + 1], f32)
            DSC = sg.tile([NLOC, NK], f32)
            nc.scalar.activation(out=DSC[:], in_=DIST[:], func=Act.Identity,
                                 bias=0.0, scale=1.0,
                                 accum_out=OUTS[0:NLOC, 4 * NGRP:4 * NGRP + 1])

            # Tsum_a = sum_p t'_ap
            ts_ps = ps_tr.tile([1, NLOC], f32, tag="tr0", name="ts_ps")
            nc.tensor.matmul(ts_ps[:], ones[:], tp[:])
            TSROW = sg.tile([1, NLOC], f32)
            nc.vector.tensor_copy(TSROW[:], ts_ps[:])
            nc.sync.dma_start(out=out2_d[:], in_=TSROW[:])

            pro_stack.close()
            wq_stack = ExitStack()
            ps_wq1 = wq_stack.enter_context(
                tc.tile_pool(name="ps_wq1", bufs=2, space="PSUM"))
            ps_wq2 = wq_stack.enter_context(
                tc.tile_pool(name="ps_wq2", bufs=2, space="PSUM"))

            # ---- main loop ----
            vb_cache = {}
            for g in range(NGRP):
                na = min(GRPA, NLOC - GRPA * g)
                nqd = (na + 7) // 8
                wq1 = ps_wq1.tile([128, 512], f32, tag="wq1", name="wq1")
                wq2 = ps_wq2.tile([128, NK2], f32, tag="wq2", name="wq2")
                dp = dpp.tile([128, NK], f32, tag="dp", name="dp")
                # dist rows of quadrant's anchors -> dp partitions 32*qd+16+s
                for qd in range(nqd):
                    bn = min(8, na - 8 * qd)
                    a0 = GRPA * g + 8 * qd
                    nc.gpsimd.dma_start(
                        out=dp[32 * qd + 16:32 * qd + 16 + bn, :],
                        in_=DIST[a0:a0 + bn, :])
                for m in range(na):
                    a = GRPA * g + m
                    qd, s8 = m // 8, m % 8
                    bn = min(8, na - 8 * qd)
                    if a % VBB == 0:
                        vb2 = vbp.tile([R, VBB, NK], bf16, tag="vb",
                                       name="vb")
                        sl = vmd[a:a + VBB, :]
                        bsrc = bass.AP(tensor=sl.tensor, offset=sl.offset,
                                       ap=[[0, R]] + [list(q) for q in sl.ap])
                        nc.sync.dma_start(out=vb2[:], in_=bsrc)
                        vb_cache[0] = vb2
                    vb = vb_cache[0][:, a % VBB, :]
                    on_act = _is_sign_anchor(a)
                    st = (s8 == 0)
                    sp = (s8 == bn - 1)
                    mt = mtp.tile([R, NK], bf16, tag="mt", name="mt")
                    if on_act:
                        nc.scalar.activation(out=mt[:], in_=vb[:],
                                             func=Act.Sign,
                                             bias=tp[0:R, a:a + 1],
                                             scale=-1.0)
                    else:
                        nc.vector.tensor_scalar(out=mt[:], in0=vb[:],
                                                scalar1=tp[0:R, a:a + 1],
                                                scalar2=None, op0=Alu.is_lt)
                    nc.tensor.matmul(wq1[32 * qd:32 * qd + 32, :],
                                     L32[0:R, :, a], mt[:, 0:512],
                                     start=st, stop=sp)
                    nc.tensor.matmul(wq2[32 * qd:32 * qd + 32, :],
                                     L32[0:R, :, a], mt[:, 512:NK],
                                     start=st, stop=sp)
                # drain group: ACT free-sums psum rows; DVE fused q*dist;
                # both accumulate straight into OUTS columns.
                sa1 = drs.tile([128, 512], f32, tag="sa1", name="sa1")
                sa2 = drs.tile([128, NK2], f32, tag="sa2", name="sa2")
                sb1 = drs.tile([128, 512], f32, tag="sb1", name="sb1")
                sb2 = drs.tile([128, NK2], f32, tag="sb2", name="sb2")
                nc.scalar.activation(out=sa1[:], in_=wq1[:], func=Act.Identity,
                                     bias=0.0, scale=1.0,
                                     accum_out=OUTS[:, 2 * g:2 * g + 1])
                nc.scalar.activation(out=sa2[:], in_=wq2[:], func=Act.Identity,
                                     bias=0.0, scale=1.0,
                                     accum_out=OUTS[:, 2 * g + 1:2 * g + 2])
                nc.vector.scalar_tensor_tensor(out=sb1[:], in0=wq1[:],
                                               scalar=1.0, in1=dp[:, 0:512],
                                               op0=Alu.mult, op1=Alu.mult,
                                               accum_out=OUTS[:, 2 * NGRP + 2 * g:
                                                              2 * NGRP + 2 * g + 1])
                nc.vector.scalar_tensor_tensor(out=sb2[:], in0=wq2[:],
                                               scalar=1.0, in1=dp[:, 512:NK],
                                               op0=Alu.mult, op1=Alu.mult,
                                               accum_out=OUTS[:, 2 * NGRP + 2 * g + 1:
                                                              2 * NGRP + 2 * g + 2])

            nc.gpsimd.dma_start(out=out_d[:], in_=OUTS[:])
            wq_stack.close()

    nc.compile()
    return nc


def _get_program(t_cls: int, nk: int, r_rows: int):
    key = ("nc", t_cls, nk, r_rows)
    if key not in _CACHE:
        _CACHE[key] = _build_program(t_cls, nk, r_rows)
    return _CACHE[key]


def _plan_layout(lab: np.ndarray):
    """Class-sort the samples; per core: the T classes its anchors span go
    into 128-aligned class tiles whose slack slots are filled with samples
    of other (non-spanned) classes, then the rest. Exactly N columns."""
    order = np.argsort(lab, kind="stable")
    slab = lab[order]
    spans = []
    t_cls = 0
    for r in range(NCORES):
        lo = slab[NLOC * r]
        hi = slab[NLOC * r + NLOC - 1]
        cls = []
        for c in range(int(lo), int(hi) + 1):
            i0 = int(np.searchsorted(slab, c, "left"))
            i1 = int(np.searchsorted(slab, c, "right"))
            if i1 > i0:
                cls.append((c, i0, i1))
                assert i1 - i0 <= 128, "class larger than 128"
        spans.append(cls)
        t_cls = max(t_cls, len(cls))
    r_rows = max(i1 - i0 for cls in spans for _, i0, i1 in cls)
    nk = N
    assert 128 * t_cls <= N, "class tiles exceed sample count"

    plans = []
    for r in range(NCORES):
        cls = spans[r]
        in_span = np.zeros(N, bool)
        pos = -np.ones(N, np.int64)      # sorted-idx -> column in core layout
        for t, (c, i0, i1) in enumerate(cls):
            in_span[i0:i1] = True
            pos[i0:i1] = 128 * t + np.arange(i1 - i0)
        rest = list(np.where(~in_span)[0])
        # fill class-tile slack with non-spanned samples (true negatives)
        for t, (c, i0, i1) in enumerate(cls):
            for p in range(i1 - i0, 128):
                pos[rest.pop()] = 128 * t + p
        nxt = 128 * len(cls)
        for sidx in rest:
            pos[sidx] = nxt
            nxt += 1
        assert nxt == N
        anchors = np.arange(NLOC * r, NLOC * r + NLOC)
        a_tile = np.zeros(NLOC, np.int64)
        a_q = np.zeros(NLOC, np.int64)
        for i, asort in enumerate(anchors):
            hit = False
            for t, (c, i0, i1) in enumerate(cls):
                if i0 <= asort < i1:
                    a_tile[i] = t
                    a_q[i] = asort - i0
                    hit = True
                    break
            assert hit, "anchor not inside its span"
        n_per_tile = [i1 - i0 for _, i0, i1 in cls]
        plans.append((pos, a_tile, a_q, n_per_tile))
    return plans, order, t_cls, nk, r_rows


def _make_inputs(embeddings: np.ndarray, labels: np.ndarray):
    e = np.ascontiguousarray(embeddings.reshape(N, D).astype(np.float32))
    lab = labels.reshape(N).astype(np.float32)
    plans, order, t_cls, nk, r_rows = _plan_layout(lab)

    in_maps = []
    for r in range(NCORES):
        pos, a_tile, a_q, n_per_tile = plans[r]
        ef = np.zeros((nk, D), np.float32)
        labr = np.zeros(nk, np.float32)
        ef[pos] = e[order]
        labr[pos] = lab[order]
        efTr = np.ascontiguousarray(ef.T)                # [D, nk]
        apos = 128 * a_tile + a_q                        # anchor columns
        poscm = np.zeros((128, NLOC), np.float32)
        for i in range(NLOC):
            nt = n_per_tile[a_tile[i]]
            poscm[:nt, i] = 1.0
            poscm[a_q[i], i] = 0.0
        selbm = np.zeros((128, t_cls * NLOC), np.float32)
        for i in range(NLOC):
            selbm[:, a_tile[i] * NLOC + i] = 1.0
        in_maps.append({
            "efT": efTr,
            "elocT": np.ascontiguousarray(efTr[:, apos]),
            "labrow": labr.reshape(1, nk),
            "llocT": np.ascontiguousarray(labr[apos].reshape(NLOC, 1)),
            "posc": poscm,
            "selb": selbm,
        })
    return in_maps, t_cls, nk, r_rows


def run_on_device(embeddings: np.ndarray, labels: np.ndarray, **run_kwargs):
    from concourse.bass_utils import run_bass_kernel_spmd
    in_maps, t_cls, nk, r_rows = _make_inputs(embeddings, labels)
    nc = _get_program(t_cls, nk, r_rows)
    res = run_bass_kernel_spmd(nc, in_maps, core_ids=list(range(NCORES)),
                               **run_kwargs)
    total = 0.0
    count = 0.0
    for r in range(NCORES):
        o = res.results[r]["out"].astype(np.float64)
        tsum = res.results[r]["out2"].astype(np.float64).reshape(-1)
        dsum = o[0:NLOC, 4 * NGRP]
        for g in range(NGRP):
            na = min(GRPA, NLOC - GRPA * g)
            for m in range(na):
                a = GRPA * g + m
                qd, s8 = m // 8, m % 8
                bw = 32 * qd + 2 * s8
                bq = 32 * qd + 16 + s8
                w = q = p2 = 0.0
                for ch in range(2):
                    w += o[bw + 0, 2 * g + ch] + o[bw + 1, 2 * g + ch]
                    q += o[bq, 2 * g + ch]
                    p2 += o[bq, 2 * NGRP + 2 * g + ch]
                if _is_sign_anchor(a):
                    w = 0.5 * w + 0.5 * nk * tsum[a]
                    q = 0.5 * q + 0.5 * r_rows * nk
                    p2 = 0.5 * p2 + 0.5 * r_rows * dsum[a]
                total += w - p2
                count += q
    return total, count, res


def kernel(embeddings: np.ndarray, labels: np.ndarray):
    embeddings = np.asarray(embeddings)
    labels = np.asarray(labels)
    total, count, _ = run_on_device(embeddings, labels)

    lab = np.asarray(labels).reshape(-1)
    cnt = np.bincount(lab.astype(np.int64), minlength=1)
    per = cnt[lab.astype(np.int64)]
    num_valid = int(((per - 1) * (N - per)).sum())

    nv = np.float32(num_valid)
    ne = np.float32(count)
    tot = np.float32(total)
    if ne > 0:
        loss = np.float32(tot / np.maximum(ne, np.float32(1.0)))
    else:
        loss = np.float32(0.0)
    frac = np.float32(ne / (nv + np.float32(1e-16)))
    return (np.array(loss, np.float32), np.array(nv, np.float32),
            np.array(ne, np.float32), np.array(frac, np.float32))
